# revision 19
# baseline (speedup 1.0000x reference)
# Trainium2 Bass kernel for the ConCH-style GNN forward pass.
# Self-contained: hardcodes shapes/sharding; host-preps inputs, runs one
# 8-core SPMD NEFF (all-gather collectives between GCN stages), gathers
# per-core outputs into full-size numpy arrays.
import os
import sys

import numpy as np

for _p in ("/root/.axon_site", "/root/.axon_site/_ro/trn_rl_repo", "/root/.axon_site/_ro/pypackages", "/opt/trn_rl_repo"):
    if _p not in sys.path and os.path.isdir(_p):
        sys.path.append(_p)

import ml_dtypes  # noqa: F401  (bf16/f16 numpy dtypes)

# problem dims
N, F_IN, H1, H2, E = 10000, 3000, 500, 64, 160000
BN_EPS = 1e-5
NCORE = 8
R = N // NCORE          # 1250 rows per core
MT = 10                 # m-tiles per core (1280 padded rows)
RP = MT * 128           # 1280
NKT = 79                # node k-tiles (79*128 = 10112 >= 10000)
NP = NKT * 128
FKT = 24                # feature k-tiles
FINP = FKT * 128
S1C = 3 * 512           # concat-S1 cols (3 encodes x 512, 500 real each)
S2C = 256               # concat-S2 cols padded (192 real)
H1P = 512

CONV_MODE = os.environ.get("KERNEL_CONV", "gather")  # "dense" | "gather"
GS = 6                  # conv1 gather group size (chunks of 128 edges)

# chunked all-gather: per-core row chunks of 2 m-tiles (256 rows), last = 226
AGC_SIZES = [256, 256, 256, 256, 226]
AGC_BASE = [0]
for _s in AGC_SIZES:
    AGC_BASE.append(AGC_BASE[-1] + NCORE * _s)   # bases in gathered node order


def _remap_nodes(g):
    """global node id -> row in the chunk-wise gathered S1/S2 tensors."""
    g = np.asarray(g, np.int64)
    core, off = g // R, g % R
    c = np.minimum(off // 256, 4)
    sizes = np.asarray(AGC_SIZES, np.int64)
    bases = np.asarray(AGC_BASE[:5], np.int64)
    return bases[c] + core * sizes[c] + (off - c * 256)


def _tile_lhsT(mat):
    """[K, M] (mults of 128) -> [M/128, 128(kp), K/128, 128(mc)] so the
    per-m DMA is per-partition contiguous."""
    K, M = mat.shape
    t = mat.reshape(K // 128, 128, M // 128, 128).transpose(2, 1, 0, 3)
    return np.ascontiguousarray(t)


def _pad2(a, k, m, dtype=np.float32):
    out = np.zeros((k, m), dtype)
    out[: a.shape[0], : a.shape[1]] = a
    return out


def _wrap_idx(idx_lin):
    """idx array (len % 16 == 0) -> [128, len/16] int16 wrapped in 16
    partitions ((ch, i) = idx[i*16+ch]) replicated across the 8 Q7 cores."""
    w = idx_lin.reshape(-1, 16).T.astype(np.int16)
    return np.ascontiguousarray(np.tile(w, (8, 1)))


def _build_nc(nch):
    import concourse.bass as bass  # noqa: F401
    import concourse.tile as tile
    from concourse import bacc, mybir, library_config
    import contextlib

    f32 = mybir.dt.float32
    f16 = mybir.dt.float16
    bf16 = mybir.dt.bfloat16
    i16 = mybir.dt.int16
    AF = mybir.ActivationFunctionType

    ng = (nch + GS - 1) // GS  # conv1 gather groups per m-tile
    nchp = ng * GS

    nc = bacc.Bacc("TRN2", target_bir_lowering=False, debug=False, num_devices=NCORE)

    # ---- dram parameters (per-core shards via in_maps) ----
    featT = nc.dram_tensor("featT", [3, MT, 128, FKT, 128], f16, kind="ExternalInput")
    w1 = nc.dram_tensor("w1", [FINP, H1], f16, kind="ExternalInput")
    w2 = nc.dram_tensor("w2", [H1P, H2], f16, kind="ExternalInput")
    wd = nc.dram_tensor("wd", [128, H1P], f16, kind="ExternalInput")
    wdec = nc.dram_tensor("wdec", [3, H1P, F_IN], f16, kind="ExternalInput")
    wbias = nc.dram_tensor("wbias", [1, 3 * F_IN], f16, kind="ExternalInput")
    discwt = nc.dram_tensor("discwt", [64, 64], f16, kind="ExternalInput")
    discb = nc.dram_tensor("discb", [1, 1], f32, kind="ExternalInput")
    gt = nc.dram_tensor("gt", [MT, 128, NKT, 128], f16, kind="ExternalInput")
    if CONV_MODE == "dense":
        at = nc.dram_tensor("at", [MT, 128, NKT, 128], f16, kind="ExternalInput")
    else:
        selt = nc.dram_tensor("selt", [MT, 128, nchp, 128], f16, kind="ExternalInput")
        gidx = nc.dram_tensor("gidx", [128, MT * nchp * 8], i16, kind="ExternalInput")

    z1o = nc.dram_tensor("z1o", [RP, H2], f32, kind="ExternalOutput")
    z2o = nc.dram_tensor("z2o", [RP, H2], f32, kind="ExternalOutput")
    z3o = nc.dram_tensor("z3o", [RP, H2], f32, kind="ExternalOutput")
    pio = nc.dram_tensor("pio", [RP, F_IN], f16, kind="ExternalOutput")
    dispo = nc.dram_tensor("dispo", [RP, F_IN], f16, kind="ExternalOutput")
    meano = nc.dram_tensor("meano", [RP, F_IN], bf16, kind="ExternalOutput")
    reco = nc.dram_tensor("reco", [RP, N], f16, kind="ExternalOutput")
    ret1t = nc.dram_tensor("ret1t", [2, RP], f32, kind="ExternalOutput")

    rg = [list(range(NCORE))]

    with tile.TileContext(nc) as tc:
        with contextlib.ExitStack() as ctx:
            dram = ctx.enter_context(tc.tile_pool(name="dram", bufs=1, space="DRAM"))
            ag1_ins = [dram.tile([AGC_SIZES[c], S1C], f16, name=f"ag1i{c}")
                       for c in range(5)]
            ag1_outs = [dram.tile([NCORE * AGC_SIZES[c], S1C], f16,
                                  addr_space="Shared", name=f"ag1o{c}")
                        for c in range(5)]
            ag1_out = dram.tile([N, S1C], f16)
            ag2_ins = [dram.tile([AGC_SIZES[c], S2C], f16, name=f"ag2i{c}")
                       for c in range(5)]
            ag2_outs = [dram.tile([NCORE * AGC_SIZES[c], S2C], f16,
                                  addr_space="Shared", name=f"ag2o{c}")
                        for c in range(5)]
            ag2_out = dram.tile([N, S2C], f16)
            ag3_in = dram.tile([R, H2], f32)
            ag3_out = dram.tile([N, H2], f32, addr_space="Shared")

            zpool = ctx.enter_context(tc.tile_pool(name="zpool", bufs=1))
            z_cat = zpool.tile([128, MT, 192], f32)

            pctx = contextlib.ExitStack()
            perm = pctx.enter_context(tc.tile_pool(name="perm", bufs=1))
            if CONV_MODE == "gather":
                nc.gpsimd.load_library(library_config.mlp)
                selt_sb = perm.tile([128, MT, nchp, 128], f16)
                for m in range(MT):
                    nc.sync.dma_start(selt_sb[:, m], selt[m])
                gidx_sb = perm.tile([128, MT * nchp * 8], i16)
                nc.sync.dma_start(gidx_sb[:], gidx[:])

            def send_chunk(c, sloc, ag_ins):
                """DMA core-local rows of ag-chunk c (m-tiles 2c, 2c+1)."""
                if c < 4:
                    nc.sync.dma_start(
                        ag_ins[c][:].rearrange("(m p) f -> p m f", p=128),
                        sloc[:, 2 * c:2 * c + 2],
                    )
                else:
                    nc.sync.dma_start(
                        ag_ins[c][:128].rearrange("(m p) f -> p m f", p=128),
                        sloc[:, 8:9],
                    )
                    nc.sync.dma_start(ag_ins[c][128:], sloc[:98, 9])

            def ag_chunk(c, ag_ins, ag_outs, ag_out):
                nc.gpsimd.collective_compute(
                    "AllGather", mybir.AluOpType.bypass,
                    ins=[ag_ins[c][:].opt()],
                    outs=[ag_outs[c][:].opt()],
                    replica_groups=rg,
                )
                nc.sync.dma_start(ag_out[AGC_BASE[c]:AGC_BASE[c + 1]], ag_outs[c][:])

            # ================= stage A: S1_e = feat_e @ W1 =================
            with tc.tile_pool(name="stA", bufs=3) as stA, \
                 tc.tile_pool(name="s1loc", bufs=1) as s1locp, \
                 tc.tile_pool(name="w1p", bufs=1) as w1p, \
                 tc.tile_pool(name="psA", bufs=2, space="PSUM") as psA:
                w1_sb = w1p.tile([128, FKT, H1], f16)
                nc.sync.dma_start(w1_sb[:], w1.ap().rearrange("(kt p) f -> p kt f", p=128))
                s1loc = s1locp.tile([128, MT, S1C], f16)
                nc.vector.memset(s1loc[:], 0.0)
                for m in range(MT):
                    for e in range(3):
                        ft = stA.tile([128, FKT, 128], f16, tag="ft")
                        nc.sync.dma_start(ft[:], featT[e, m])
                        ps = psA.tile([128, H1], f32, tag="psA")
                        for kt in range(FKT):
                            nc.tensor.matmul(ps[:], ft[:, kt], w1_sb[:, kt],
                                             start=(kt == 0), stop=(kt == FKT - 1))
                        nc.scalar.activation(s1loc[:, m, e * 512:e * 512 + H1], ps[:], AF.Copy)
                    if m % 2 == 1 and m < 9:
                        send_chunk(m // 2, s1loc, ag1_ins)
                        ag_chunk(m // 2, ag1_ins, ag1_outs, ag1_out)
                    elif m == 9:
                        send_chunk(4, s1loc, ag1_ins)
                        ag_chunk(4, ag1_ins, ag1_outs, ag1_out)

            # ================= stage B: H = relu(A @ S1) =================
            hctx = contextlib.ExitStack()
            hpool = hctx.enter_context(tc.tile_pool(name="hpool", bufs=1))
            h_es = [hpool.tile([128, MT, 512], f16, tag=f"h{e}", name=f"h{e}") for e in range(3)]
            if CONV_MODE == "dense":
                with tc.tile_pool(name="stB", bufs=2) as stB, \
                     tc.tile_pool(name="s1f", bufs=1) as s1fp, \
                     tc.tile_pool(name="psB", bufs=2, space="PSUM") as psB:
                    for e in range(3):
                        s1f = s1fp.tile([128, NKT, 512], f16, tag="s1f")
                        nc.vector.memset(s1f[:, NKT - 1], 0.0)
                        src = ag1_out[:, e * 512:(e + 1) * 512]
                        nc.sync.dma_start(
                            s1f[:, : NKT - 1],
                            src[: (NKT - 1) * 128].rearrange("(kt p) f -> p kt f", p=128),
                        )
                        nc.sync.dma_start(s1f[:16, NKT - 1], src[(NKT - 1) * 128:])
                        for m in range(MT):
                            a0 = stB.tile([128, 40, 128], f16, tag="at")
                            a1 = stB.tile([128, NKT - 40, 128], f16, tag="at2")
                            nc.sync.dma_start(a0[:], at[m, :, :40])
                            nc.sync.dma_start(a1[:], at[m, :, 40:])
                            ps = psB.tile([128, 512], f32, tag="psB")
                            for kt in range(NKT):
                                lhs = a0[:, kt] if kt < 40 else a1[:, kt - 40]
                                nc.tensor.matmul(ps[:], lhs, s1f[:, kt],
                                                 start=(kt == 0), stop=(kt == NKT - 1))
                            nc.scalar.activation(h_es[e][:, m], ps[:], AF.Relu)
            else:
                with tc.tile_pool(name="stB", bufs=3) as stB, \
                     tc.tile_pool(name="psB", bufs=2, space="PSUM") as psB:
                    for m in range(MT):
                        ps = psB.tile([128, S1C], f32, tag="psB")
                        for g in range(ng):
                            gb = stB.tile([128, GS, S1C], f16, tag="gb")
                            nc.gpsimd.dma_gather(
                                out_ap=gb[:], in_ap=ag1_out[:],
                                idxs_ap=gidx_sb[:, (m * ng + g) * GS * 8:(m * ng + g + 1) * GS * 8],
                                num_idxs=GS * 128, num_idxs_reg=GS * 128, elem_size=S1C,
                            )
                            for h in range(GS):
                                ch = g * GS + h
                                for c in range(3):
                                    nc.tensor.matmul(
                                        ps[:, c * 512:(c + 1) * 512],
                                        selt_sb[:, m, ch], gb[:, h, c * 512:(c + 1) * 512],
                                        start=(ch == 0), stop=(ch == nchp - 1),
                                    )
                        for e in range(3):
                            nc.scalar.activation(h_es[e][:, m], ps[:, e * 512:(e + 1) * 512], AF.Relu)

            # ============ stage C: HT (dma transpose), S2 = H @ W2 ============
            with tc.tile_pool(name="stC", bufs=2) as stC, \
                 tc.tile_pool(name="s2loc", bufs=1) as s2locp, \
                 tc.tile_pool(name="psC", bufs=2, space="PSUM") as psC:
                w2_sb = stC.tile([128, 4, H2], f16, tag="w2")
                nc.sync.dma_start(w2_sb[:], w2.ap().rearrange("(kt p) f -> p kt f", p=128))
                s2loc = s2locp.tile([128, MT, S2C], f16)
                nc.vector.memset(s2loc[:, :, 192:], 0.0)
                hts = []
                for e in range(3):
                    ht = stC.tile([128, MT, 4, 128], f16, tag=f"ht{e}", name=f"ht{e}")
                    # ht[d, m, fb, p] = h_es[e][p, m, fb*128 + d]
                    nc.scalar.dma_start(ht[:], h_es[e][:], transpose=True)
                    hts.append(ht)
                for m in range(MT):
                    for e in range(3):
                        ps = psC.tile([128, H2], f32, tag="psC")
                        for kc in range(4):
                            nc.tensor.matmul(ps[:], hts[e][:, m, kc], w2_sb[:, kc],
                                             start=(kc == 0), stop=(kc == 3))
                        nc.scalar.activation(s2loc[:, m, e * 64:(e + 1) * 64], ps[:], AF.Copy)
                    if m % 2 == 1 and m < 9:
                        send_chunk(m // 2, s2loc, ag2_ins)
                        ag_chunk(m // 2, ag2_ins, ag2_outs, ag2_out)
                    elif m == 9:
                        send_chunk(4, s2loc, ag2_ins)
                        ag_chunk(4, ag2_ins, ag2_outs, ag2_out)
            hctx.close()

            # ================= stage D: Z = A @ S2 =================
            if CONV_MODE == "dense":
                with tc.tile_pool(name="stD", bufs=3) as stD, \
                     tc.tile_pool(name="s2f", bufs=1) as s2fp, \
                     tc.tile_pool(name="psD", bufs=2, space="PSUM") as psD:
                    s2f = s2fp.tile([128, NKT, 192], f16)
                    nc.vector.memset(s2f[:, NKT - 1], 0.0)
                    src = ag2_out[:, :192]
                    nc.sync.dma_start(
                        s2f[:, : NKT - 1],
                        src[: (NKT - 1) * 128].rearrange("(kt p) f -> p kt f", p=128),
                    )
                    nc.sync.dma_start(s2f[:16, NKT - 1], src[(NKT - 1) * 128:])
                    for m in range(MT):
                        a0 = stD.tile([128, 40, 128], f16, tag="at")
                        a1 = stD.tile([128, NKT - 40, 128], f16, tag="at2")
                        nc.sync.dma_start(a0[:], at[m, :, :40])
                        nc.sync.dma_start(a1[:], at[m, :, 40:])
                        ps = psD.tile([128, 192], f32, tag="psD")
                        for kt in range(NKT):
                            lhs = a0[:, kt] if kt < 40 else a1[:, kt - 40]
                            nc.tensor.matmul(ps[:], lhs, s2f[:, kt],
                                             start=(kt == 0), stop=(kt == NKT - 1))
                        nc.scalar.activation(z_cat[:, m], ps[:], AF.Copy)
            else:
                with tc.tile_pool(name="stD", bufs=2) as stD, \
                     tc.tile_pool(name="psD", bufs=2, space="PSUM") as psD:
                    for m in range(MT):
                        ps = psD.tile([128, S2C], f32, tag="psD")
                        for g in range(ng):
                            gb = stD.tile([128, GS, S2C], f16, tag="gb2")
                            nc.gpsimd.dma_gather(
                                out_ap=gb[:], in_ap=ag2_out[:],
                                idxs_ap=gidx_sb[:, (m * ng + g) * GS * 8:(m * ng + g + 1) * GS * 8],
                                num_idxs=GS * 128, num_idxs_reg=GS * 128, elem_size=S2C,
                            )
                            for h in range(GS):
                                ch = g * GS + h
                                nc.tensor.matmul(ps[:], selt_sb[:, m, ch], gb[:, h],
                                                 start=(ch == 0), stop=(ch == nchp - 1))
                        nc.scalar.activation(z_cat[:, m], ps[:, :192], AF.Copy)

            pctx.close()
            # z outputs + z1 all-gather
            nc.sync.dma_start(z1o[:].rearrange("(m p) f -> p m f", p=128), z_cat[:, :, 0:64])
            nc.sync.dma_start(z2o[:].rearrange("(m p) f -> p m f", p=128), z_cat[:, :, 64:128])
            nc.sync.dma_start(z3o[:].rearrange("(m p) f -> p m f", p=128), z_cat[:, :, 128:192])
            nc.sync.dma_start(
                ag3_in[: 9 * 128].rearrange("(m p) f -> p m f", p=128),
                z_cat[:, :9, 0:64],
            )
            nc.sync.dma_start(ag3_in[9 * 128:], z_cat[:98, 9, 0:64])
            nc.gpsimd.collective_compute(
                "AllGather", mybir.AluOpType.bypass,
                ins=[ag3_in[:].opt()], outs=[ag3_out[:].opt()], replica_groups=rg,
            )

            # ======= stage E: z1 full: relu + l2norm + transposes =======
            epool = ctx.enter_context(tc.tile_pool(name="epool", bufs=1))
            e1f = epool.tile([128, NKT, H2], f16)         # relu(z1_full), readout rhs
            znt = epool.tile([128, NKT, 128], f16)        # zn_full^T (rows 0:64 valid)
            zlt = epool.tile([128, MT, 128], f16)         # zn_local^T
            z1ta = epool.tile([128, MT, 128], f16)        # z1_local^T + ones row
            e1t = epool.tile([128, MT, 128], f16)         # emb1_local^T
            e3t = epool.tile([128, MT, 128], f16)         # emb3_local^T
            g2t = epool.tile([128, MT, 128], f16)         # g2^T
            with tc.tile_pool(name="stE", bufs=1) as stE:
                z1f = stE.tile([128, NKT, H2], f32, tag="z1f")
                nc.vector.memset(z1f[:, NKT - 1], 0.0)
                nc.sync.dma_start(
                    z1f[:, : NKT - 1],
                    ag3_out[: (NKT - 1) * 128].rearrange("(kt p) f -> p kt f", p=128),
                )
                nc.sync.dma_start(z1f[:16, NKT - 1], ag3_out[(NKT - 1) * 128:])
                nc.scalar.activation(e1f[:], z1f[:], AF.Relu)
                # row l2 norms of z1_full
                sq = stE.tile([128, NKT, H2], f32, tag="sq")
                nc.scalar.activation(sq[:], z1f[:], AF.Square)
                nrm = stE.tile([128, NKT], f32, tag="nrm")
                nc.vector.tensor_reduce(nrm[:], sq[:], mybir.AxisListType.X, mybir.AluOpType.add)
                nc.vector.tensor_scalar_max(nrm[:], nrm[:], 1e-24)
                nc.scalar.activation(nrm[:], nrm[:], AF.Sqrt)
                nc.vector.reciprocal(nrm[:], nrm[:])
                znp = stE.tile([128, NKT, 128], f16, tag="znp")
                nc.vector.memset(znp[:, :, 64:], 0.0)
                for kt in range(NKT):
                    nc.vector.tensor_scalar_mul(znp[:, kt, 0:64], z1f[:, kt], nrm[:, kt:kt + 1])
                nc.scalar.dma_start(znt[:], znp[:], transpose=True)

                # local transposes: z1 (with ones row), zn_local, emb1, emb3
                lp = stE.tile([128, MT, 128], f16, tag="lp")
                nc.vector.memset(lp[:, :, 64:], 0.0)
                nc.vector.tensor_copy(lp[:, :, 0:64], z_cat[:, :, 0:64])
                nc.scalar.dma_start(z1ta[:], lp[:], transpose=True)
                nc.vector.memset(z1ta[64:65], 1.0)
                sql = stE.tile([128, MT, H2], f32, tag="sql")
                nc.scalar.activation(sql[:], z_cat[:, :, 0:64], AF.Square)
                nrml = stE.tile([128, MT], f32, tag="nrml")
                nc.vector.tensor_reduce(nrml[:], sql[:], mybir.AxisListType.X, mybir.AluOpType.add)
                nc.vector.tensor_scalar_max(nrml[:], nrml[:], 1e-24)
                nc.scalar.activation(nrml[:], nrml[:], AF.Sqrt)
                nc.vector.reciprocal(nrml[:], nrml[:])
                lp2 = stE.tile([128, MT, 128], f16, tag="lp2")
                nc.vector.memset(lp2[:, :, 64:], 0.0)
                for m in range(MT):
                    nc.vector.tensor_scalar_mul(lp2[:, m, 0:64], z_cat[:, m, 0:64], nrml[:, m:m + 1])
                nc.scalar.dma_start(zlt[:], lp2[:], transpose=True)
                lp3 = stE.tile([128, MT, 128], f16, tag="lp3")
                nc.vector.memset(lp3[:, :, 64:], 0.0)
                nc.scalar.activation(lp3[:, :, 0:64], z_cat[:, :, 0:64], AF.Relu)
                nc.scalar.dma_start(e1t[:], lp3[:], transpose=True)
                lp4 = stE.tile([128, MT, 128], f16, tag="lp4")
                nc.vector.memset(lp4[:, :, 64:], 0.0)
                nc.scalar.activation(lp4[:, :, 0:64], z_cat[:, :, 128:192], AF.Relu)
                nc.scalar.dma_start(e3t[:], lp4[:], transpose=True)

            # ================= stage F: rec_adj =================
            zntv = znt[:].rearrange("p a b -> p (a b)")
            zltv = zlt[:].rearrange("p a b -> p (a b)")
            ncols = [512] * 19 + [272]
            with tc.tile_pool(name="stF", bufs=2) as stF, \
                 tc.tile_pool(name="psF", bufs=2, space="PSUM") as psF:
                for m in range(MT):
                    rstage = stF.tile([128, N], f16, tag="rstage")
                    c0 = 0
                    for w in ncols:
                        ps = psF.tile([128, 512], f32, tag="psF")
                        nc.tensor.matmul(ps[:, :w], zltv[0:64, m * 128:(m + 1) * 128],
                                         zntv[0:64, c0:c0 + w])
                        nc.scalar.activation(rstage[:, c0:c0 + w], ps[:, :w], AF.Sigmoid)
                        c0 += w
                    nc.sync.dma_start(reco[m * 128:(m + 1) * 128], rstage[:])

            # ================= stage G: readout + g2 =================
            with tc.tile_pool(name="stG", bufs=2) as stG, \
                 tc.tile_pool(name="psG", bufs=2, space="PSUM") as psG:
                vsum = stG.tile([128, MT, H2], f32, tag="vsum")
                nrmg = stG.tile([128, MT], f32, tag="nrmg")
                sqg = stG.tile([128, MT, H2], f32, tag="sqg")
                for m in range(MT):
                    g0 = stG.tile([128, 40, 128], f16, tag="gt")
                    g1 = stG.tile([128, NKT - 40, 128], f16, tag="gt2")
                    nc.sync.dma_start(g0[:], gt[m, :, :40])
                    nc.sync.dma_start(g1[:], gt[m, :, 40:])
                    ps = psG.tile([128, H2], f32, tag="psG")
                    for kt in range(NKT):
                        lhs = g0[:, kt] if kt < 40 else g1[:, kt - 40]
                        nc.tensor.matmul(ps[:], lhs, e1f[:, kt],
                                         start=(kt == 0), stop=(kt == NKT - 1))
                    nc.scalar.activation(vsum[:, m], ps[:], AF.Copy)
                    nc.scalar.activation(sqg[:, m], ps[:], AF.Square)
                nc.vector.tensor_reduce(nrmg[:], sqg[:], mybir.AxisListType.X, mybir.AluOpType.add)
                nc.vector.tensor_scalar_max(nrmg[:], nrmg[:], 1e-24)
                nc.scalar.activation(nrmg[:], nrmg[:], AF.Sqrt)
                nc.vector.reciprocal(nrmg[:], nrmg[:])
                g2p = stG.tile([128, MT, 128], f16, tag="g2p")
                nc.vector.memset(g2p[:, :, 64:], 0.0)
                for m in range(MT):
                    nc.vector.tensor_scalar_mul(g2p[:, m, 0:64], vsum[:, m], nrmg[:, m:m + 1])
                nc.scalar.activation(g2p[:, :, 0:64], g2p[:, :, 0:64], AF.Sigmoid)
                nc.scalar.dma_start(g2t[:], g2p[:], transpose=True)

            # ================= stage H: discriminator =================
            with tc.tile_pool(name="stH", bufs=1) as stH, \
                 tc.tile_pool(name="psH", bufs=2, space="PSUM") as psH:
                dwt = stH.tile([64, 64], f16, tag="dwt")
                nc.sync.dma_start(dwt[:], discwt[:])
                dbs = stH.tile([1, 1], f32, tag="dbs")
                nc.sync.dma_start(dbs[:], discb[:])
                ones64 = stH.tile([64, 1], f16, tag="ones64")
                nc.vector.memset(ones64[:], 1.0)
                g2tv = g2t[:].rearrange("p a b -> p (a b)")
                tmpt = stH.tile([64, RP], f16, tag="tmpt")
                for c0 in range(0, RP, 512):
                    w = min(512, RP - c0)
                    ps = psH.tile([64, 512], f32, tag="psH1")
                    nc.tensor.matmul(ps[:, :w], dwt[:], g2tv[0:64, c0:c0 + w])
                    nc.scalar.activation(tmpt[:, c0:c0 + w], ps[:, :w], AF.Copy)
                for si, ebuf in ((0, e1t), (1, e3t)):
                    scrow = stH.tile([1, RP], f32, tag=f"scrow{si}", name=f"scrow{si}")
                    ebv = ebuf[:].rearrange("p a b -> p (a b)")
                    prod = stH.tile([64, RP], f16, tag="prod")
                    nc.vector.tensor_mul(prod[:], ebv[0:64], tmpt[:])
                    for c0 in range(0, RP, 512):
                        w = min(512, RP - c0)
                        ps = psH.tile([1, 512], f32, tag="psH2")
                        nc.tensor.matmul(ps[:, :w], ones64[:], prod[:, c0:c0 + w])
                        nc.scalar.activation(scrow[0:1, c0:c0 + w], ps[:, :w], AF.Identity,
                                             bias=dbs[0:1, 0:1])
                    nc.sync.dma_start(ret1t[si:si + 1], scrow[:])

            # ================= stage I: ZINB decoder =================
            with tc.tile_pool(name="stI1", bufs=1) as stI1, \
                 tc.tile_pool(name="stg", bufs=3) as stgp, \
                 tc.tile_pool(name="wdecp", bufs=2) as wdecp, \
                 tc.tile_pool(name="psI", bufs=2, space="PSUM") as psI:
                # xdT = relu(bn-folded(Wd' @ z1T_aug))
                wd_sb = stI1.tile([128, H1P], f16, tag="wd")
                nc.sync.dma_start(wd_sb[:], wd[:])
                xdt = stI1.tile([128, 4, RP], f16, tag="xdt")
                z1tav = z1ta[:].rearrange("p a b -> p (a b)")
                for ft in range(4):
                    for c0 in range(0, RP, 512):
                        w = min(512, RP - c0)
                        ps = psI.tile([128, 512], f32, tag="psI")
                        nc.tensor.matmul(ps[:, :w], wd_sb[:, ft * 128:(ft + 1) * 128],
                                         z1tav[:, c0:c0 + w])
                        nc.scalar.activation(xdt[:, ft, c0:c0 + w], ps[:, :w], AF.Relu)
                wb_sb = stI1.tile([1, 3 * F_IN], f16, tag="wb")
                nc.sync.dma_start(wb_sb[:], wbias[:])
                ones1 = stI1.tile([1, 128], f16, tag="ones1")
                nc.vector.memset(ones1[:], 1.0)
                dcols = [512] * 5 + [440]
                outs = (pio, dispo, meano)
                for h in range(3):
                    wdec_sb = wdecp.tile([128, 4, F_IN], f16, tag="wdech")
                    nc.sync.dma_start(
                        wdec_sb[:], wdec[h].rearrange("(kt p) f -> p kt f", p=128)
                    )
                    for m in range(MT):
                        stg = stgp.tile([128, F_IN], bf16 if h == 2 else f16,
                                        tag=f"stg{int(h == 2)}", name=f"stg{int(h == 2)}")
                        c0 = 0
                        for w in dcols:
                            ps = psI.tile([128, 512], f32, tag="psI")
                            for kc in range(4):
                                nc.tensor.matmul(ps[:, :w], xdt[:, kc, m * 128:(m + 1) * 128],
                                                 wdec_sb[:, kc, c0:c0 + w],
                                                 start=(kc == 0), stop=False)
                            nc.tensor.matmul(ps[:, :w], ones1[:],
                                             wb_sb[0:1, h * F_IN + c0:h * F_IN + c0 + w],
                                             start=False, stop=True)
                            if h == 0:
                                nc.scalar.activation(stg[:, c0:c0 + w], ps[:, :w], AF.Sigmoid)
                            elif h == 1:
                                exf = stgp.tile([128, 512], f32, tag="exf")
                                nc.scalar.activation(exf[:, :w], ps[:, :w], AF.Exp)
                                nc.scalar.activation(stg[:, c0:c0 + w], exf[:, :w], AF.Ln, bias=1.0)
                            else:
                                nc.scalar.activation(stg[:, c0:c0 + w], ps[:, :w], AF.Exp)
                            c0 += w
                        if h == 1:
                            nc.vector.tensor_scalar_min(stg[:], stg[:], 1e4)
                            nc.vector.tensor_scalar_max(stg[:], stg[:], 1e-4)
                        elif h == 2:
                            nc.vector.tensor_scalar_min(stg[:], stg[:], 1e6)
                            nc.vector.tensor_scalar_max(stg[:], stg[:], 1e-5)
                        nc.sync.dma_start(outs[h][m * 128:(m + 1) * 128], stg[:])

    nc.compile()
    return nc


def _host_prep(inputs):
    """Build per-core in_maps + conv chunk count (gather mode)."""
    feat = np.asarray(inputs["feat"], np.float32)
    feat_a = np.asarray(inputs["feat_a"], np.float32)
    feat_b = np.asarray(inputs["feat_b"], np.float32)
    rows = np.asarray(inputs["adj_rows"]).astype(np.int64)
    cols = np.asarray(inputs["adj_cols"]).astype(np.int64)
    vals = np.asarray(inputs["adj_vals"], np.float32)
    gneigh = np.asarray(inputs["graph_neigh"], np.float32)
    W1 = np.asarray(inputs["W1"], np.float32)
    W2 = np.asarray(inputs["W2"], np.float32)
    Wd = np.asarray(inputs["Wd"], np.float32)
    bd = np.asarray(inputs["bd"], np.float32)
    bn_gamma = np.asarray(inputs["bn_gamma"], np.float32)
    bn_beta = np.asarray(inputs["bn_beta"], np.float32)
    bn_mean = np.asarray(inputs["bn_mean"], np.float32)
    bn_var = np.asarray(inputs["bn_var"], np.float32)
    Wpi = np.asarray(inputs["Wpi"], np.float32)
    bpi = np.asarray(inputs["bpi"], np.float32)
    Wdisp = np.asarray(inputs["Wdisp"], np.float32)
    bdisp = np.asarray(inputs["bdisp"], np.float32)
    Wmean = np.asarray(inputs["Wmean"], np.float32)
    bmean = np.asarray(inputs["bmean"], np.float32)
    disc_W = np.asarray(inputs["disc_W"], np.float32)
    disc_b = np.float32(inputs["disc_b"])

    # shared (replicated) weights
    w1_u = _pad2(W1, FINP, H1).astype(np.float16)
    w2_u = _pad2(W2, H1P, H2).astype(np.float16)
    scale = bn_gamma / np.sqrt(bn_var + BN_EPS)
    wd_aug = np.zeros((128, H1P), np.float32)
    wd_aug[:H2, :H1] = Wd * scale[None, :]
    wd_aug[H2, :H1] = (bd - bn_mean) * scale + bn_beta
    wd_u = wd_aug.astype(np.float16)
    wdec_u = np.stack([_pad2(Wpi, H1P, F_IN), _pad2(Wdisp, H1P, F_IN),
                       _pad2(Wmean, H1P, F_IN)]).astype(np.float16)
    wbias_u = np.concatenate([bpi, bdisp, bmean]).reshape(1, -1).astype(np.float16)
    discwt_u = np.ascontiguousarray(disc_W.T).astype(np.float16)
    discb_u = np.full((1, 1), disc_b, np.float32)

    cols_r = _remap_nodes(cols)     # source nodes in chunked-AG row order

    maps = []
    for ci in range(NCORE):
        r0, r1 = ci * R, (ci + 1) * R
        m = {}
        ftl = []
        for f in (feat, feat_a, feat_b):
            fT = _pad2(np.ascontiguousarray(f[r0:r1].T), FINP, RP)
            ftl.append(_tile_lhsT(fT))
        m["featT"] = np.stack(ftl).astype(np.float16)
        gT = _pad2(np.ascontiguousarray(gneigh[r0:r1].T), NP, RP)
        m["gt"] = _tile_lhsT(gT).astype(np.float16)
        m.update(w1=w1_u, w2=w2_u, wd=wd_u, wdec=wdec_u, wbias=wbias_u,
                 discwt=discwt_u, discb=discb_u)
        maps.append(m)

    if CONV_MODE == "dense":
        AT = np.zeros((N, N), np.float32)
        np.add.at(AT, (cols_r, rows), vals)  # rows of AT in chunked-AG order
        for ci in range(NCORE):
            r0, r1 = ci * R, (ci + 1) * R
            atc = _pad2(AT[:, r0:r1], NP, RP)
            maps[ci]["at"] = _tile_lhsT(atc).astype(np.float16)
        return maps, 0

    # gather mode: per-core edge lists grouped by dest m-tile, chunked by 128
    core_of = rows // R
    per_core = []
    nch = 0
    for ci in range(NCORE):
        sel = core_of == ci
        r = (rows[sel] - ci * R).astype(np.int64)
        c = cols_r[sel]
        v = vals[sel]
        mt = r // 128
        buckets = []
        for m in range(MT):
            ms = mt == m
            buckets.append((r[ms] - m * 128, c[ms], v[ms]))
            nch = max(nch, (int(ms.sum()) + 127) // 128)
        per_core.append(buckets)
    # fixed chunk count across cores & m-tiles (SPMD: one program)
    ng = (nch + GS - 1) // GS
    nchp = ng * GS
    for ci in range(NCORE):
        selt = np.zeros((MT, nchp, 128, 128), np.float32)  # [m, ch, edge_slot, row]
        idx = np.zeros((MT, nchp * 128), np.int64)
        for m in range(MT):
            rl, c, v = per_core[ci][m]
            ne = len(rl)
            idx[m, :ne] = c
            ch = np.arange(ne) // 128
            slot = np.arange(ne) % 128
            selt[m, ch, slot, rl] = v
        selt = np.ascontiguousarray(selt.transpose(0, 2, 1, 3))
        maps[ci]["selt"] = selt.astype(np.float16)
        gi = np.zeros((128, MT * nchp * 8), np.int16)
        for m in range(MT):
            gi[:, m * nchp * 8:(m + 1) * nchp * 8] = _wrap_idx(idx[m])
        maps[ci]["gidx"] = gi
    return maps, nch


_BUILD_CACHE = {}


def kernel(**inputs):
    from concourse.bass_utils import run_bass_kernel_spmd

    in_maps, nch = _host_prep(inputs)
    key = (CONV_MODE, nch)
    if key not in _BUILD_CACHE:
        _BUILD_CACHE[key] = _build_nc(nch)
    nc = _BUILD_CACHE[key]

    res = run_bass_kernel_spmd(nc, in_maps, core_ids=list(range(NCORE)),
                               trace=False)
    outs = res.results

    def cat(name):
        return np.concatenate(
            [np.asarray(outs[ci][name][:R], np.float32) for ci in range(NCORE)], axis=0
        )

    z1 = cat("z1o")
    z2 = cat("z2o")
    z3 = cat("z3o")
    pi = cat("pio")
    disp = cat("dispo")
    mean = cat("meano")
    rec = cat("reco")
    ret1 = np.concatenate(
        [np.asarray(outs[ci]["ret1t"], np.float32)[:, :R].T for ci in range(NCORE)],
        axis=0,
    )
    return z1, z2, z3, pi, disp, mean, rec, ret1


# revision 21
# speedup vs baseline: 1.0924x; 1.0924x over previous
# Trainium2 Bass kernel for the ConCH-style GNN forward pass.
# Self-contained: hardcodes shapes/sharding; host-preps inputs, runs one
# 8-core SPMD NEFF (all-gather collectives between GCN stages), gathers
# per-core outputs into full-size numpy arrays.
import os
import sys

import numpy as np

for _p in ("/root/.axon_site", "/root/.axon_site/_ro/trn_rl_repo", "/root/.axon_site/_ro/pypackages", "/opt/trn_rl_repo"):
    if _p not in sys.path and os.path.isdir(_p):
        sys.path.append(_p)

import ml_dtypes  # noqa: F401  (bf16/f16 numpy dtypes)

# problem dims
N, F_IN, H1, H2, E = 10000, 3000, 500, 64, 160000
BN_EPS = 1e-5
NCORE = 8
R = N // NCORE          # 1250 rows per core
MT = 10                 # m-tiles per core (1280 padded rows)
RP = MT * 128           # 1280
NKT = 79                # node k-tiles (79*128 = 10112 >= 10000)
NP = NKT * 128
FKT = 24                # feature k-tiles
FINP = FKT * 128
S1C = 3 * 512           # concat-S1 cols (3 encodes x 512, 500 real each)
S2C = 256               # concat-S2 cols padded (192 real)
H1P = 512

CONV_MODE = os.environ.get("KERNEL_CONV", "gather")  # "dense" | "gather"
GS = 6                  # conv1 gather group size (chunks of 128 edges)

# chunked all-gather: per-core row chunks of 2 m-tiles (256 rows), last = 226
AGC_SIZES = [256, 256, 256, 256, 226]
AGC_BASE = [0]
for _s in AGC_SIZES:
    AGC_BASE.append(AGC_BASE[-1] + NCORE * _s)   # bases in gathered node order


def _remap_nodes(g):
    """global node id -> row in the chunk-wise gathered S1/S2 tensors."""
    g = np.asarray(g, np.int64)
    core, off = g // R, g % R
    c = np.minimum(off // 256, 4)
    sizes = np.asarray(AGC_SIZES, np.int64)
    bases = np.asarray(AGC_BASE[:5], np.int64)
    return bases[c] + core * sizes[c] + (off - c * 256)


def _tile_lhsT(mat):
    """[K, M] (mults of 128) -> [M/128, 128(kp), K/128, 128(mc)] so the
    per-m DMA is per-partition contiguous."""
    K, M = mat.shape
    t = mat.reshape(K // 128, 128, M // 128, 128).transpose(2, 1, 0, 3)
    return np.ascontiguousarray(t)


def _pad2(a, k, m, dtype=np.float32):
    out = np.zeros((k, m), dtype)
    out[: a.shape[0], : a.shape[1]] = a
    return out


def _wrap_idx(idx_lin):
    """idx array (len % 16 == 0) -> [128, len/16] int16 wrapped in 16
    partitions ((ch, i) = idx[i*16+ch]) replicated across the 8 Q7 cores."""
    w = idx_lin.reshape(-1, 16).T.astype(np.int16)
    return np.ascontiguousarray(np.tile(w, (8, 1)))


def _build_nc(nch, bounds):
    import concourse.bass as bass  # noqa: F401
    import concourse.tile as tile
    from concourse import bacc, mybir, library_config
    import contextlib

    f32 = mybir.dt.float32
    f16 = mybir.dt.float16
    bf16 = mybir.dt.bfloat16
    i16 = mybir.dt.int16
    AF = mybir.ActivationFunctionType

    ng = (nch + GS - 1) // GS  # conv1 gather groups per m-tile
    nchp = ng * GS

    nc = bacc.Bacc("TRN2", target_bir_lowering=False, debug=False, num_devices=NCORE)

    # ---- dram parameters (per-core shards via in_maps) ----
    featT = nc.dram_tensor("featT", [3, MT, 128, FKT, 128], f16, kind="ExternalInput")
    w1 = nc.dram_tensor("w1", [FINP, H1], f16, kind="ExternalInput")
    w2 = nc.dram_tensor("w2", [H1P, H2], f16, kind="ExternalInput")
    wd = nc.dram_tensor("wd", [128, H1P], f16, kind="ExternalInput")
    wdec = nc.dram_tensor("wdec", [3, H1P, F_IN], f16, kind="ExternalInput")
    wbias = nc.dram_tensor("wbias", [1, 3 * F_IN], f16, kind="ExternalInput")
    discwt = nc.dram_tensor("discwt", [64, 64], f16, kind="ExternalInput")
    discb = nc.dram_tensor("discb", [1, 1], f32, kind="ExternalInput")
    gt = nc.dram_tensor("gt", [MT, 128, NKT, 128], f16, kind="ExternalInput")
    at2 = nc.dram_tensor("at2", [MT, 128, NKT, 128], f16, kind="ExternalInput")
    if CONV_MODE == "dense":
        at = nc.dram_tensor("at", [MT, 128, NKT, 128], f16, kind="ExternalInput")
    else:
        selt = nc.dram_tensor("selt", [MT, 128, nchp, 128], f16, kind="ExternalInput")
        gidx = nc.dram_tensor("gidx", [128, MT * nchp * 8], i16, kind="ExternalInput")

    z1o = nc.dram_tensor("z1o", [RP, H2], f32, kind="ExternalOutput")
    z2o = nc.dram_tensor("z2o", [RP, H2], f32, kind="ExternalOutput")
    z3o = nc.dram_tensor("z3o", [RP, H2], f32, kind="ExternalOutput")
    pio = nc.dram_tensor("pio", [RP, F_IN], f16, kind="ExternalOutput")
    dispo = nc.dram_tensor("dispo", [RP, F_IN], f16, kind="ExternalOutput")
    meano = nc.dram_tensor("meano", [RP, F_IN], bf16, kind="ExternalOutput")
    reco = nc.dram_tensor("reco", [RP, N], f16, kind="ExternalOutput")
    ret1t = nc.dram_tensor("ret1t", [2, RP], f32, kind="ExternalOutput")

    rg = [list(range(NCORE))]

    with tile.TileContext(nc) as tc:
        with contextlib.ExitStack() as ctx:
            dram = ctx.enter_context(tc.tile_pool(name="dram", bufs=1, space="DRAM"))
            ag1_ins = [dram.tile([AGC_SIZES[c], S1C], f16, name=f"ag1i{c}")
                       for c in range(5)]
            ag1_outs = [dram.tile([NCORE * AGC_SIZES[c], S1C], f16,
                                  addr_space="Shared", name=f"ag1o{c}")
                        for c in range(5)]
            ag1_out = dram.tile([N, S1C], f16)
            ag2_in = dram.tile([R, S2C], f16)
            ag2_out = dram.tile([N, S2C], f16, addr_space="Shared")
            ag3_in = dram.tile([R, H2], f32)
            ag3_out = dram.tile([N, H2], f32, addr_space="Shared")

            zpool = ctx.enter_context(tc.tile_pool(name="zpool", bufs=1))
            z_cat = zpool.tile([128, MT, 192], f32)
            s2loc_g = zpool.tile([128, MT, S2C], f16)
            s2loc_ref = [s2loc_g]

            pctx = contextlib.ExitStack()
            perm = pctx.enter_context(tc.tile_pool(name="perm", bufs=1))
            if CONV_MODE == "gather":
                nc.gpsimd.load_library(library_config.mlp)
                selt_sb = perm.tile([128, MT, nchp, 128], f16)
                for m in range(MT):
                    nc.sync.dma_start(selt_sb[:, m], selt[m])
                gidx_sb = perm.tile([128, MT * nchp * 8], i16)
                nc.sync.dma_start(gidx_sb[:], gidx[:])

            def send_chunk(c, sloc, ag_ins):
                """DMA core-local rows of ag-chunk c (m-tiles 2c, 2c+1)."""
                if c < 4:
                    nc.sync.dma_start(
                        ag_ins[c][:].rearrange("(m p) f -> p m f", p=128),
                        sloc[:, 2 * c:2 * c + 2],
                    )
                else:
                    nc.sync.dma_start(
                        ag_ins[c][:128].rearrange("(m p) f -> p m f", p=128),
                        sloc[:, 8:9],
                    )
                    nc.sync.dma_start(ag_ins[c][128:], sloc[:98, 9])

            def ag_chunk(c, ag_ins, ag_outs, ag_out):
                nc.gpsimd.collective_compute(
                    "AllGather", mybir.AluOpType.bypass,
                    ins=[ag_ins[c][:].opt()],
                    outs=[ag_outs[c][:].opt()],
                    replica_groups=rg,
                )
                nc.sync.dma_start(ag_out[AGC_BASE[c]:AGC_BASE[c + 1]], ag_outs[c][:])

            # ================= stage A: S1_e = feat_e @ W1 =================
            with tc.tile_pool(name="stA", bufs=3) as stA, \
                 tc.tile_pool(name="s1loc", bufs=1) as s1locp, \
                 tc.tile_pool(name="w1p", bufs=1) as w1p, \
                 tc.tile_pool(name="psA", bufs=2, space="PSUM") as psA:
                w1_sb = w1p.tile([128, FKT, H1], f16)
                nc.sync.dma_start(w1_sb[:], w1.ap().rearrange("(kt p) f -> p kt f", p=128))
                s1loc = s1locp.tile([128, MT, S1C], f16)
                nc.vector.memset(s1loc[:], 0.0)
                for m in range(MT):
                    for e in range(3):
                        ft = stA.tile([128, FKT, 128], f16, tag="ft")
                        nc.sync.dma_start(ft[:], featT[e, m])
                        ps = psA.tile([128, H1], f32, tag="psA")
                        for kt in range(FKT):
                            nc.tensor.matmul(ps[:], ft[:, kt], w1_sb[:, kt],
                                             start=(kt == 0), stop=(kt == FKT - 1))
                        nc.scalar.activation(s1loc[:, m, e * 512:e * 512 + H1], ps[:], AF.Copy)
                    if m % 2 == 1 and m < 9:
                        send_chunk(m // 2, s1loc, ag1_ins)
                        ag_chunk(m // 2, ag1_ins, ag1_outs, ag1_out)
                    elif m == 9:
                        send_chunk(4, s1loc, ag1_ins)
                        ag_chunk(4, ag1_ins, ag1_outs, ag1_out)

            # ================= stage B: H = relu(A @ S1) =================
            hctx = contextlib.ExitStack()
            hpool = hctx.enter_context(tc.tile_pool(name="hpool", bufs=1))
            h_es = [hpool.tile([128, MT, 512], f16, tag=f"h{e}", name=f"h{e}") for e in range(3)]
            if CONV_MODE == "dense":
                with tc.tile_pool(name="stB", bufs=2) as stB, \
                     tc.tile_pool(name="s1f", bufs=1) as s1fp, \
                     tc.tile_pool(name="psB", bufs=2, space="PSUM") as psB:
                    for e in range(3):
                        s1f = s1fp.tile([128, NKT, 512], f16, tag="s1f")
                        nc.vector.memset(s1f[:, NKT - 1], 0.0)
                        src = ag1_out[:, e * 512:(e + 1) * 512]
                        nc.sync.dma_start(
                            s1f[:, : NKT - 1],
                            src[: (NKT - 1) * 128].rearrange("(kt p) f -> p kt f", p=128),
                        )
                        nc.sync.dma_start(s1f[:16, NKT - 1], src[(NKT - 1) * 128:])
                        for m in range(MT):
                            a0 = stB.tile([128, 40, 128], f16, tag="at")
                            a1 = stB.tile([128, NKT - 40, 128], f16, tag="at2")
                            nc.sync.dma_start(a0[:], at[m, :, :40])
                            nc.sync.dma_start(a1[:], at[m, :, 40:])
                            ps = psB.tile([128, 512], f32, tag="psB")
                            for kt in range(NKT):
                                lhs = a0[:, kt] if kt < 40 else a1[:, kt - 40]
                                nc.tensor.matmul(ps[:], lhs, s1f[:, kt],
                                                 start=(kt == 0), stop=(kt == NKT - 1))
                            nc.scalar.activation(h_es[e][:, m], ps[:], AF.Relu)
            else:
                with tc.tile_pool(name="stB", bufs=3) as stB, \
                     tc.tile_pool(name="psB", bufs=2, space="PSUM") as psB:
                    for m in range(MT):
                        ps = psB.tile([128, S1C], f32, tag="psB")
                        for g in range(ng):
                            gb = stB.tile([128, GS, S1C], f16, tag="gb")
                            nc.gpsimd.dma_gather(
                                out_ap=gb[:], in_ap=ag1_out[:bounds[g]],
                                idxs_ap=gidx_sb[:, (m * ng + g) * GS * 8:(m * ng + g + 1) * GS * 8],
                                num_idxs=GS * 128, num_idxs_reg=GS * 128, elem_size=S1C,
                            )
                            for h in range(GS):
                                ch = g * GS + h
                                for c in range(3):
                                    nc.tensor.matmul(
                                        ps[:, c * 512:(c + 1) * 512],
                                        selt_sb[:, m, ch], gb[:, h, c * 512:(c + 1) * 512],
                                        start=(ch == 0), stop=(ch == nchp - 1),
                                    )
                        for e in range(3):
                            nc.scalar.activation(h_es[e][:, m], ps[:, e * 512:(e + 1) * 512], AF.Relu)

            # ============ stage C: HT (dma transpose), S2 = H @ W2 ============
            with tc.tile_pool(name="stC", bufs=2) as stC, \
                 tc.tile_pool(name="psC", bufs=2, space="PSUM") as psC:
                w2_sb = stC.tile([128, 4, H2], f16, tag="w2")
                nc.sync.dma_start(w2_sb[:], w2.ap().rearrange("(kt p) f -> p kt f", p=128))
                s2loc = s2loc_g
                nc.vector.memset(s2loc[:, :, 192:], 0.0)
                hts = []
                for e in range(3):
                    ht = stC.tile([128, MT, 4, 128], f16, tag=f"ht{e}", name=f"ht{e}")
                    # ht[d, m, fb, p] = h_es[e][p, m, fb*128 + d]
                    nc.scalar.dma_start(ht[:], h_es[e][:], transpose=True)
                    hts.append(ht)
                for m in range(MT):
                    for e in range(3):
                        ps = psC.tile([128, H2], f32, tag="psC")
                        for kc in range(4):
                            nc.tensor.matmul(ps[:], hts[e][:, m, kc], w2_sb[:, kc],
                                             start=(kc == 0), stop=(kc == 3))
                        nc.scalar.activation(s2loc[:, m, e * 64:(e + 1) * 64], ps[:], AF.Copy)
            hctx.close()
            nc.sync.dma_start(
                ag2_in[: 9 * 128].rearrange("(m p) f -> p m f", p=128),
                s2loc_ref[0][:, :9],
            )
            nc.sync.dma_start(ag2_in[9 * 128:], s2loc_ref[0][:98, 9])
            nc.gpsimd.collective_compute(
                "AllGather", mybir.AluOpType.bypass,
                ins=[ag2_in[:].opt()], outs=[ag2_out[:].opt()], replica_groups=rg,
            )

            # ================= stage D: Z = A @ S2 (dense) =================
            with tc.tile_pool(name="stD", bufs=3) as stD, \
                 tc.tile_pool(name="s2f", bufs=1) as s2fp, \
                 tc.tile_pool(name="psD", bufs=2, space="PSUM") as psD:
                s2f = s2fp.tile([128, NKT, 192], f16)
                nc.vector.memset(s2f[:, NKT - 1], 0.0)
                src2 = ag2_out[:, :192]
                nc.sync.dma_start(
                    s2f[:, : NKT - 1],
                    src2[: (NKT - 1) * 128].rearrange("(kt p) f -> p kt f", p=128),
                )
                nc.sync.dma_start(s2f[:16, NKT - 1], src2[(NKT - 1) * 128:])
                for m in range(MT):
                    a0 = stD.tile([128, 40, 128], f16, tag="at")
                    a1 = stD.tile([128, NKT - 40, 128], f16, tag="at2")
                    nc.sync.dma_start(a0[:], at2[m, :, :40])
                    nc.sync.dma_start(a1[:], at2[m, :, 40:])
                    ps = psD.tile([128, 192], f32, tag="psD")
                    for kt in range(NKT):
                        lhs = a0[:, kt] if kt < 40 else a1[:, kt - 40]
                        nc.tensor.matmul(ps[:], lhs, s2f[:, kt],
                                         start=(kt == 0), stop=(kt == NKT - 1))
                    nc.scalar.activation(z_cat[:, m], ps[:], AF.Copy)

            pctx.close()
            # z outputs + z1 all-gather
            nc.sync.dma_start(z1o[:].rearrange("(m p) f -> p m f", p=128), z_cat[:, :, 0:64])
            nc.sync.dma_start(z2o[:].rearrange("(m p) f -> p m f", p=128), z_cat[:, :, 64:128])
            nc.sync.dma_start(z3o[:].rearrange("(m p) f -> p m f", p=128), z_cat[:, :, 128:192])
            nc.sync.dma_start(
                ag3_in[: 9 * 128].rearrange("(m p) f -> p m f", p=128),
                z_cat[:, :9, 0:64],
            )
            nc.sync.dma_start(ag3_in[9 * 128:], z_cat[:98, 9, 0:64])
            nc.gpsimd.collective_compute(
                "AllGather", mybir.AluOpType.bypass,
                ins=[ag3_in[:].opt()], outs=[ag3_out[:].opt()], replica_groups=rg,
            )

            # ======= stage E: z1 full: relu + l2norm + transposes =======
            epool = ctx.enter_context(tc.tile_pool(name="epool", bufs=1))
            e1f = epool.tile([128, NKT, H2], f16)         # relu(z1_full), readout rhs
            znt = epool.tile([128, NKT, 128], f16)        # zn_full^T (rows 0:64 valid)
            zlt = epool.tile([128, MT, 128], f16)         # zn_local^T
            z1ta = epool.tile([128, MT, 128], f16)        # z1_local^T + ones row
            e1t = epool.tile([128, MT, 128], f16)         # emb1_local^T
            e3t = epool.tile([128, MT, 128], f16)         # emb3_local^T
            g2t = epool.tile([128, MT, 128], f16)         # g2^T
            with tc.tile_pool(name="stE", bufs=1) as stE:
                z1f = stE.tile([128, NKT, H2], f32, tag="z1f")
                nc.vector.memset(z1f[:, NKT - 1], 0.0)
                nc.sync.dma_start(
                    z1f[:, : NKT - 1],
                    ag3_out[: (NKT - 1) * 128].rearrange("(kt p) f -> p kt f", p=128),
                )
                nc.sync.dma_start(z1f[:16, NKT - 1], ag3_out[(NKT - 1) * 128:])
                nc.scalar.activation(e1f[:], z1f[:], AF.Relu)
                # row l2 norms of z1_full
                sq = stE.tile([128, NKT, H2], f32, tag="sq")
                nc.scalar.activation(sq[:], z1f[:], AF.Square)
                nrm = stE.tile([128, NKT], f32, tag="nrm")
                nc.vector.tensor_reduce(nrm[:], sq[:], mybir.AxisListType.X, mybir.AluOpType.add)
                nc.vector.tensor_scalar_max(nrm[:], nrm[:], 1e-24)
                nc.scalar.activation(nrm[:], nrm[:], AF.Ln)
                nc.scalar.activation(nrm[:], nrm[:], AF.Exp, scale=-0.5)
                znp = stE.tile([128, NKT, 128], f16, tag="znp")
                nc.vector.memset(znp[:, :, 64:], 0.0)
                for kt in range(NKT):
                    nc.vector.tensor_scalar_mul(znp[:, kt, 0:64], z1f[:, kt], nrm[:, kt:kt + 1])
                nc.scalar.dma_start(znt[:], znp[:], transpose=True)

                # local transposes: z1 (with ones row), zn_local, emb1, emb3
                lp = stE.tile([128, MT, 128], f16, tag="lp")
                nc.vector.memset(lp[:, :, 64:], 0.0)
                nc.vector.tensor_copy(lp[:, :, 0:64], z_cat[:, :, 0:64])
                nc.scalar.dma_start(z1ta[:], lp[:], transpose=True)
                nc.vector.memset(z1ta[64:65], 1.0)
                sql = stE.tile([128, MT, H2], f32, tag="sql")
                nc.scalar.activation(sql[:], z_cat[:, :, 0:64], AF.Square)
                nrml = stE.tile([128, MT], f32, tag="nrml")
                nc.vector.tensor_reduce(nrml[:], sql[:], mybir.AxisListType.X, mybir.AluOpType.add)
                nc.vector.tensor_scalar_max(nrml[:], nrml[:], 1e-24)
                nc.scalar.activation(nrml[:], nrml[:], AF.Ln)
                nc.scalar.activation(nrml[:], nrml[:], AF.Exp, scale=-0.5)
                lp2 = stE.tile([128, MT, 128], f16, tag="lp2")
                nc.vector.memset(lp2[:, :, 64:], 0.0)
                for m in range(MT):
                    nc.vector.tensor_scalar_mul(lp2[:, m, 0:64], z_cat[:, m, 0:64], nrml[:, m:m + 1])
                nc.scalar.dma_start(zlt[:], lp2[:], transpose=True)
                lp3 = stE.tile([128, MT, 128], f16, tag="lp3")
                nc.vector.memset(lp3[:, :, 64:], 0.0)
                nc.scalar.activation(lp3[:, :, 0:64], z_cat[:, :, 0:64], AF.Relu)
                nc.scalar.dma_start(e1t[:], lp3[:], transpose=True)
                lp4 = stE.tile([128, MT, 128], f16, tag="lp4")
                nc.vector.memset(lp4[:, :, 64:], 0.0)
                nc.scalar.activation(lp4[:, :, 0:64], z_cat[:, :, 128:192], AF.Relu)
                nc.scalar.dma_start(e3t[:], lp4[:], transpose=True)

            # ================= stage F: rec_adj =================
            zntv = znt[:].rearrange("p a b -> p (a b)")
            zltv = zlt[:].rearrange("p a b -> p (a b)")
            ncols = [512] * 19 + [272]
            with tc.tile_pool(name="stF", bufs=2) as stF, \
                 tc.tile_pool(name="psF", bufs=2, space="PSUM") as psF:
                for m in range(MT):
                    rstage = stF.tile([128, N], f16, tag="rstage")
                    c0 = 0
                    for w in ncols:
                        ps = psF.tile([128, 512], f32, tag="psF")
                        nc.tensor.matmul(ps[:, :w], zltv[0:64, m * 128:(m + 1) * 128],
                                         zntv[0:64, c0:c0 + w])
                        nc.scalar.activation(rstage[:, c0:c0 + w], ps[:, :w], AF.Sigmoid)
                        c0 += w
                    nc.sync.dma_start(reco[m * 128:(m + 1) * 128], rstage[:])

            # ================= stage G: readout + g2 =================
            with tc.tile_pool(name="stG", bufs=2) as stG, \
                 tc.tile_pool(name="psG", bufs=2, space="PSUM") as psG:
                vsum = stG.tile([128, MT, H2], f32, tag="vsum")
                nrmg = stG.tile([128, MT], f32, tag="nrmg")
                sqg = stG.tile([128, MT, H2], f32, tag="sqg")
                for m in range(MT):
                    g0 = stG.tile([128, 40, 128], f16, tag="gt")
                    g1 = stG.tile([128, NKT - 40, 128], f16, tag="gt2")
                    nc.sync.dma_start(g0[:], gt[m, :, :40])
                    nc.sync.dma_start(g1[:], gt[m, :, 40:])
                    ps = psG.tile([128, H2], f32, tag="psG")
                    for kt in range(NKT):
                        lhs = g0[:, kt] if kt < 40 else g1[:, kt - 40]
                        nc.tensor.matmul(ps[:], lhs, e1f[:, kt],
                                         start=(kt == 0), stop=(kt == NKT - 1))
                    nc.scalar.activation(vsum[:, m], ps[:], AF.Copy)
                    nc.scalar.activation(sqg[:, m], ps[:], AF.Square)
                nc.vector.tensor_reduce(nrmg[:], sqg[:], mybir.AxisListType.X, mybir.AluOpType.add)
                nc.vector.tensor_scalar_max(nrmg[:], nrmg[:], 1e-24)
                nc.scalar.activation(nrmg[:], nrmg[:], AF.Ln)
                nc.scalar.activation(nrmg[:], nrmg[:], AF.Exp, scale=-0.5)
                g2p = stG.tile([128, MT, 128], f16, tag="g2p")
                nc.vector.memset(g2p[:, :, 64:], 0.0)
                for m in range(MT):
                    nc.vector.tensor_scalar_mul(g2p[:, m, 0:64], vsum[:, m], nrmg[:, m:m + 1])
                nc.scalar.activation(g2p[:, :, 0:64], g2p[:, :, 0:64], AF.Sigmoid)
                nc.scalar.dma_start(g2t[:], g2p[:], transpose=True)

            # ================= stage H: discriminator =================
            with tc.tile_pool(name="stH", bufs=1) as stH, \
                 tc.tile_pool(name="psH", bufs=2, space="PSUM") as psH:
                dwt = stH.tile([64, 64], f16, tag="dwt")
                nc.sync.dma_start(dwt[:], discwt[:])
                dbs = stH.tile([1, 1], f32, tag="dbs")
                nc.sync.dma_start(dbs[:], discb[:])
                ones64 = stH.tile([64, 1], f16, tag="ones64")
                nc.vector.memset(ones64[:], 1.0)
                g2tv = g2t[:].rearrange("p a b -> p (a b)")
                tmpt = stH.tile([64, RP], f16, tag="tmpt")
                for c0 in range(0, RP, 512):
                    w = min(512, RP - c0)
                    ps = psH.tile([64, 512], f32, tag="psH1")
                    nc.tensor.matmul(ps[:, :w], dwt[:], g2tv[0:64, c0:c0 + w])
                    nc.scalar.activation(tmpt[:, c0:c0 + w], ps[:, :w], AF.Copy)
                for si, ebuf in ((0, e1t), (1, e3t)):
                    scrow = stH.tile([1, RP], f32, tag=f"scrow{si}", name=f"scrow{si}")
                    ebv = ebuf[:].rearrange("p a b -> p (a b)")
                    prod = stH.tile([64, RP], f16, tag="prod")
                    nc.vector.tensor_mul(prod[:], ebv[0:64], tmpt[:])
                    for c0 in range(0, RP, 512):
                        w = min(512, RP - c0)
                        ps = psH.tile([1, 512], f32, tag="psH2")
                        nc.tensor.matmul(ps[:, :w], ones64[:], prod[:, c0:c0 + w])
                        nc.scalar.activation(scrow[0:1, c0:c0 + w], ps[:, :w], AF.Identity,
                                             bias=dbs[0:1, 0:1])
                    nc.sync.dma_start(ret1t[si:si + 1], scrow[:])

            # ================= stage I: ZINB decoder =================
            with tc.tile_pool(name="stI1", bufs=1) as stI1, \
                 tc.tile_pool(name="stg", bufs=3) as stgp, \
                 tc.tile_pool(name="wdecp", bufs=2) as wdecp, \
                 tc.tile_pool(name="psI", bufs=2, space="PSUM") as psI:
                # xdT = relu(bn-folded(Wd' @ z1T_aug))
                wd_sb = stI1.tile([128, H1P], f16, tag="wd")
                nc.sync.dma_start(wd_sb[:], wd[:])
                xdt = stI1.tile([128, 4, RP], f16, tag="xdt")
                z1tav = z1ta[:].rearrange("p a b -> p (a b)")
                for ft in range(4):
                    for c0 in range(0, RP, 512):
                        w = min(512, RP - c0)
                        ps = psI.tile([128, 512], f32, tag="psI")
                        nc.tensor.matmul(ps[:, :w], wd_sb[:, ft * 128:(ft + 1) * 128],
                                         z1tav[:, c0:c0 + w])
                        nc.scalar.activation(xdt[:, ft, c0:c0 + w], ps[:, :w], AF.Relu)
                wb_sb = stI1.tile([1, 3 * F_IN], f16, tag="wb")
                nc.sync.dma_start(wb_sb[:], wbias[:])
                ones1 = stI1.tile([1, 128], f16, tag="ones1")
                nc.vector.memset(ones1[:], 1.0)
                dcols = [512] * 5 + [440]
                outs = (pio, dispo, meano)
                for h in range(3):
                    wdec_sb = wdecp.tile([128, 4, F_IN], f16, tag="wdech")
                    nc.sync.dma_start(
                        wdec_sb[:], wdec[h].rearrange("(kt p) f -> p kt f", p=128)
                    )
                    for m in range(MT):
                        stg = stgp.tile([128, F_IN], bf16 if h == 2 else f16,
                                        tag=f"stg{int(h == 2)}", name=f"stg{int(h == 2)}")
                        c0 = 0
                        for w in dcols:
                            ps = psI.tile([128, 512], f32, tag="psI")
                            for kc in range(4):
                                nc.tensor.matmul(ps[:, :w], xdt[:, kc, m * 128:(m + 1) * 128],
                                                 wdec_sb[:, kc, c0:c0 + w],
                                                 start=(kc == 0), stop=False)
                            nc.tensor.matmul(ps[:, :w], ones1[:],
                                             wb_sb[0:1, h * F_IN + c0:h * F_IN + c0 + w],
                                             start=False, stop=True)
                            if h == 0:
                                nc.scalar.activation(stg[:, c0:c0 + w], ps[:, :w], AF.Sigmoid)
                            elif h == 1:
                                exf = stgp.tile([128, 512], f32, tag="exf")
                                nc.scalar.activation(exf[:, :w], ps[:, :w], AF.Exp)
                                nc.scalar.activation(stg[:, c0:c0 + w], exf[:, :w], AF.Ln, bias=1.0)
                            else:
                                nc.scalar.activation(stg[:, c0:c0 + w], ps[:, :w], AF.Exp)
                            c0 += w
                        if h == 1:
                            nc.vector.tensor_scalar_min(stg[:], stg[:], 1e4)
                            nc.vector.tensor_scalar_max(stg[:], stg[:], 1e-4)
                        elif h == 2:
                            nc.vector.tensor_scalar_min(stg[:], stg[:], 1e6)
                            nc.vector.tensor_scalar_max(stg[:], stg[:], 1e-5)
                        nc.sync.dma_start(outs[h][m * 128:(m + 1) * 128], stg[:])

    nc.compile()
    return nc


def _host_prep(inputs):
    """Build per-core in_maps + conv chunk count (gather mode)."""
    feat = np.asarray(inputs["feat"], np.float32)
    feat_a = np.asarray(inputs["feat_a"], np.float32)
    feat_b = np.asarray(inputs["feat_b"], np.float32)
    rows = np.asarray(inputs["adj_rows"]).astype(np.int64)
    cols = np.asarray(inputs["adj_cols"]).astype(np.int64)
    vals = np.asarray(inputs["adj_vals"], np.float32)
    gneigh = np.asarray(inputs["graph_neigh"], np.float32)
    W1 = np.asarray(inputs["W1"], np.float32)
    W2 = np.asarray(inputs["W2"], np.float32)
    Wd = np.asarray(inputs["Wd"], np.float32)
    bd = np.asarray(inputs["bd"], np.float32)
    bn_gamma = np.asarray(inputs["bn_gamma"], np.float32)
    bn_beta = np.asarray(inputs["bn_beta"], np.float32)
    bn_mean = np.asarray(inputs["bn_mean"], np.float32)
    bn_var = np.asarray(inputs["bn_var"], np.float32)
    Wpi = np.asarray(inputs["Wpi"], np.float32)
    bpi = np.asarray(inputs["bpi"], np.float32)
    Wdisp = np.asarray(inputs["Wdisp"], np.float32)
    bdisp = np.asarray(inputs["bdisp"], np.float32)
    Wmean = np.asarray(inputs["Wmean"], np.float32)
    bmean = np.asarray(inputs["bmean"], np.float32)
    disc_W = np.asarray(inputs["disc_W"], np.float32)
    disc_b = np.float32(inputs["disc_b"])

    # shared (replicated) weights
    w1_u = _pad2(W1, FINP, H1).astype(np.float16)
    w2_u = _pad2(W2, H1P, H2).astype(np.float16)
    scale = bn_gamma / np.sqrt(bn_var + BN_EPS)
    wd_aug = np.zeros((128, H1P), np.float32)
    wd_aug[:H2, :H1] = Wd * scale[None, :]
    wd_aug[H2, :H1] = (bd - bn_mean) * scale + bn_beta
    wd_u = wd_aug.astype(np.float16)
    wdec_u = np.stack([_pad2(Wpi, H1P, F_IN), _pad2(Wdisp, H1P, F_IN),
                       _pad2(Wmean, H1P, F_IN)]).astype(np.float16)
    wbias_u = np.concatenate([bpi, bdisp, bmean]).reshape(1, -1).astype(np.float16)
    discwt_u = np.ascontiguousarray(disc_W.T).astype(np.float16)
    discb_u = np.full((1, 1), disc_b, np.float32)

    cols_r = _remap_nodes(cols)     # source nodes in chunked-AG row order

    maps = []
    for ci in range(NCORE):
        r0, r1 = ci * R, (ci + 1) * R
        m = {}
        ftl = []
        for f in (feat, feat_a, feat_b):
            fT = _pad2(np.ascontiguousarray(f[r0:r1].T), FINP, RP)
            ftl.append(_tile_lhsT(fT))
        m["featT"] = np.stack(ftl).astype(np.float16)
        gT = _pad2(np.ascontiguousarray(gneigh[r0:r1].T), NP, RP)
        m["gt"] = _tile_lhsT(gT).astype(np.float16)
        m.update(w1=w1_u, w2=w2_u, wd=wd_u, wdec=wdec_u, wbias=wbias_u,
                 discwt=discwt_u, discb=discb_u)
        maps.append(m)

    ATn = np.zeros((N, N), np.float32)
    np.add.at(ATn, (cols, rows), vals)       # natural order (conv2)
    for ci in range(NCORE):
        r0, r1 = ci * R, (ci + 1) * R
        maps[ci]["at2"] = _tile_lhsT(_pad2(ATn[:, r0:r1], NP, RP)).astype(np.float16)
    if CONV_MODE == "dense":
        AT = np.zeros((N, N), np.float32)
        np.add.at(AT, (cols_r, rows), vals)  # rows of AT in chunked-AG order
        for ci in range(NCORE):
            r0, r1 = ci * R, (ci + 1) * R
            atc = _pad2(AT[:, r0:r1], NP, RP)
            maps[ci]["at"] = _tile_lhsT(atc).astype(np.float16)
        return maps, 0, ()

    # gather mode: per-core edge lists grouped by dest m-tile, chunked by 128
    core_of = rows // R
    per_core = []
    nch = 0
    for ci in range(NCORE):
        sel = core_of == ci
        r = (rows[sel] - ci * R).astype(np.int64)
        c = cols_r[sel]
        v = vals[sel]
        mt = r // 128
        buckets = []
        for m in range(MT):
            ms = mt == m
            rl, cl, vl = r[ms] - m * 128, c[ms], v[ms]
            order = np.argsort(cl, kind="stable")
            buckets.append((rl[order], cl[order], vl[order]))
            nch = max(nch, (int(ms.sum()) + 127) // 128)
        per_core.append(buckets)
    # fixed chunk count across cores & m-tiles (SPMD: one program)
    ng = (nch + GS - 1) // GS
    nchp = ng * GS
    bounds = np.full(ng, 128, np.int64)
    for ci in range(NCORE):
        selt = np.zeros((MT, nchp, 128, 128), np.float32)  # [m, ch, edge_slot, row]
        idx = np.zeros((MT, nchp * 128), np.int64)
        for m in range(MT):
            rl, c, v = per_core[ci][m]
            ne = len(rl)
            idx[m, :ne] = c
            ch = np.arange(ne) // 128
            slot = np.arange(ne) % 128
            selt[m, ch, slot, rl] = v
            for g in range(ng):
                seg = c[g * GS * 128:(g + 1) * GS * 128]
                if len(seg):
                    bounds[g] = max(bounds[g], int(seg.max()) + 1)
        selt = np.ascontiguousarray(selt.transpose(0, 2, 1, 3))
        maps[ci]["selt"] = selt.astype(np.float16)
        gi = np.zeros((128, MT * nchp * 8), np.int16)
        for m in range(MT):
            gi[:, m * nchp * 8:(m + 1) * nchp * 8] = _wrap_idx(idx[m])
        maps[ci]["gidx"] = gi
    return maps, nch, tuple(int(b) for b in bounds)


_BUILD_CACHE = {}


def kernel(**inputs):
    from concourse.bass_utils import run_bass_kernel_spmd

    in_maps, nch, bounds = _host_prep(inputs)
    key = (CONV_MODE, nch, bounds)
    if key not in _BUILD_CACHE:
        _BUILD_CACHE[key] = _build_nc(nch, bounds)
    nc = _BUILD_CACHE[key]

    res = run_bass_kernel_spmd(nc, in_maps, core_ids=list(range(NCORE)),
                               trace=False)
    outs = res.results

    def cat(name):
        return np.concatenate(
            [np.asarray(outs[ci][name][:R], np.float32) for ci in range(NCORE)], axis=0
        )

    z1 = cat("z1o")
    z2 = cat("z2o")
    z3 = cat("z3o")
    pi = cat("pio")
    disp = cat("dispo")
    mean = cat("meano")
    rec = cat("reco")
    ret1 = np.concatenate(
        [np.asarray(outs[ci]["ret1t"], np.float32)[:, :R].T for ci in range(NCORE)],
        axis=0,
    )
    return z1, z2, z3, pi, disp, mean, rec, ret1
